# revision 11
# baseline (speedup 1.0000x reference)
"""CLUB loss kernel for Trainium2 (8 NeuronCores, SPMD row-sharded).

Math: the reference returns mean_i(pos_i - neg_i), a scalar.  Expanding
both terms, the C = sum mu^2*invv piece cancels exactly between pos and
neg, so the kernel only needs six fused reduction passes:

  loss = -0.5/N * (A - 2B) + 0.5/N^2 * (S_invv . S_x2 - 2 S_muinvv . S_x)
      A        = sum_{i,d} x^2 * invv        (scalar)
      B        = sum_{i,d} x * mu * invv     (scalar)
      S_invv   = sum_i invv[i,:]             (d-vector)
      S_muinvv = sum_i mu[i,:]*invv[i,:]     (d-vector)
      S_x      = sum_j x[j,:]                (d-vector)
      S_x2     = sum_j x[j,:]^2              (d-vector)

Each core handles 2048 rows (2 batches of x + matching mu/logvar rows)
and emits f32 partial sums; the host combines them in float64.

Layout: d-major (128, 1024): partition q = (sub-slab b, dim d), free
axis = row index.  Every reduction is a free-axis row-sum riding the
accum_out port of a fused elementwise op.

Schedule: asymmetric column halves a=[0:640], b=[640:1024].  mu ships
as ONE full-width fp16 tensor (halves the bytes at 2KB DMA lines;
quantization costs ~3e-3 end-to-end, gate is 2e-2).  Queues:
  sync:   lv_a, x_a          scalar: lv_b, mu, x_b      (640KB each)
ACT runs exp (small half first) + squares + Sx_a copy; DVE runs the
muinvv/B/A chain + Sx_b.  DVE junk outputs share one tile, WAW-pinning
their order against the scheduler's optimistic DMA model.
"""

import sys

sys.path.insert(0, "/opt/trn_rl_repo")

import numpy as np
from contextlib import ExitStack

import concourse.bass as bass
import concourse.bacc as bacc
import concourse.tile as tile
from concourse import mybir
from concourse.bass_utils import run_bass_kernel_spmd

F32 = mybir.dt.float32
F16 = mybir.dt.float16
BF16 = mybir.dt.bfloat16
N_CORES = 8
B, D, H, W = 16, 64, 32, 32
HW = H * W                # 1024
N = B * HW                # 16384
NB = B // N_CORES         # 2 sub-slabs (batches) per core
ROWS = NB * HW            # 2048 rows per core
COLS = HW                 # free size of the (128, 1024) layout

SPLIT = 640
SL = [slice(0, SPLIT), slice(SPLIT, COLS)]

# accumulator columns: [Sinvv a,b | Smuinvv a,b | Sx2 a,b | Sx a,b |
#                       A a,b | B a,b]
ACC_COLS = 12


def _col(acc, base, i):
    return acc[:, base + i:base + i + 1]


def build_nc() -> bass.Bass:
    nc = bacc.Bacc()
    lvt = [nc.dram_tensor(f"lv{h}", [128, s.stop - s.start], F32,
                          kind="ExternalInput") for h, s in enumerate(SL)]
    mut = nc.dram_tensor("muf", [128, COLS], F16, kind="ExternalInput")
    xt = [nc.dram_tensor(f"x{h}", [128, s.stop - s.start], F32,
                         kind="ExternalInput") for h, s in enumerate(SL)]
    accs = nc.dram_tensor("accs", [128, ACC_COLS], F32, kind="ExternalOutput")

    M = mybir.AluOpType.mult
    ADD = mybir.AluOpType.add
    BYP = mybir.AluOpType.bypass
    EXP = mybir.ActivationFunctionType.Exp
    SQ = mybir.ActivationFunctionType.Square
    CP = mybir.ActivationFunctionType.Copy

    with ExitStack() as ctx:
        tc = ctx.enter_context(tile.TileContext(nc))
        big = ctx.enter_context(tc.tile_pool(name="big", bufs=1))
        accp = ctx.enter_context(tc.tile_pool(name="accp", bufs=1))

        xb = big.tile([128, COLS], F32)
        mu = big.tile([128, COLS], F16)
        lv = big.tile([128, COLS], F32)
        invv = big.tile([128, COLS], F32)
        muinvv = big.tile([128, COLS], F32)
        x2 = big.tile([128, COLS], F32)
        jd = big.tile([128, SPLIT], BF16)   # shared junk out: WAW-chains DVE order
        ja = big.tile([128, SPLIT], BF16)   # ACT junk out for the Sx_a copy
        acc = accp.tile([128, ACC_COLS], F32)

        nc.sync.dma_start(out=lv[:, SL[0]], in_=lvt[0][:, :])
        nc.scalar.dma_start(out=lv[:, SL[1]], in_=lvt[1][:, :])
        nc.sync.dma_start(out=xb[:, SL[0]], in_=xt[0][:, :])
        nc.scalar.dma_start(out=mu[:, :], in_=mut[:, :])
        nc.scalar.dma_start(out=xb[:, SL[1]], in_=xt[1][:, :])

        # ACT: exp of the small half first (its lv lands first), then the
        # big half, squares in x-arrival order, then the Sx_a copy.
        for h in (1, 0):
            s = SL[h]
            nc.scalar.activation(
                out=invv[:, s], in_=lv[:, s], func=EXP, bias=0.0, scale=-1.0,
                accum_out=_col(acc, 0, h),
            )
        for h in (0, 1):
            s = SL[h]
            nc.scalar.activation(
                out=x2[:, s], in_=xb[:, s], func=SQ, bias=0.0, scale=1.0,
                accum_out=_col(acc, 4, h),
            )
        nc.scalar.activation(
            out=ja[:], in_=xb[:, SL[0]], func=CP, bias=0.0, scale=1.0,
            accum_out=_col(acc, 6, 0),
        )

        # DVE: muinvv_b (mu lands before x), muinvv_a, then the jd-chained
        # B/A per half and Sx_b.
        nc.vector.scalar_tensor_tensor(
            out=muinvv[:, SL[1]], in0=mu[:, SL[1]], scalar=1.0,
            in1=invv[:, SL[1]], op0=M, op1=M, accum_out=_col(acc, 2, 1),
        )
        nc.vector.scalar_tensor_tensor(
            out=muinvv[:, SL[0]], in0=mu[:, SL[0]], scalar=1.0,
            in1=invv[:, SL[0]], op0=M, op1=M, accum_out=_col(acc, 2, 0),
        )
        nc.vector.scalar_tensor_tensor(
            out=jd[:], in0=xb[:, SL[0]], scalar=1.0, in1=muinvv[:, SL[0]],
            op0=M, op1=M, accum_out=_col(acc, 10, 0),  # B_a
        )
        nc.vector.scalar_tensor_tensor(
            out=jd[:], in0=x2[:, SL[0]], scalar=1.0, in1=invv[:, SL[0]],
            op0=M, op1=M, accum_out=_col(acc, 8, 0),   # A_a
        )
        nc.vector.scalar_tensor_tensor(
            out=jd[:, :COLS - SPLIT], in0=xb[:, SL[1]], scalar=1.0,
            in1=muinvv[:, SL[1]], op0=M, op1=M,
            accum_out=_col(acc, 10, 1),                # B_b
        )
        nc.vector.scalar_tensor_tensor(
            out=jd[:, :COLS - SPLIT], in0=x2[:, SL[1]], scalar=1.0,
            in1=invv[:, SL[1]], op0=M, op1=M,
            accum_out=_col(acc, 8, 1),                 # A_b
        )
        nc.vector.tensor_scalar(
            out=jd[:, :COLS - SPLIT], in0=xb[:, SL[1]], scalar1=1.0,
            scalar2=0.0, op0=M, op1=ADD, accum_out=_col(acc, 6, 1),  # Sx_b
        )

        nc.sync.dma_start(out=accs[:, :], in_=acc[:])
    return nc


def _ensure_ntff_hook():
    """This image's antenv lacks axon_hooks; if tracing is requested
    (e.g. BASS_TRACE=1), run_bass_kernel_spmd would die on the import.
    Register the ctypes-based hook if available, else a None hook so
    tracing degrades gracefully."""
    import types

    if "antenv.axon_hooks" in sys.modules:
        return
    try:
        import antenv.axon_hooks  # noqa: F401
        return
    except ImportError:
        pass
    hook = None
    try:
        sys.path.insert(0, "/root/.axon_site")
        from trn_agent_boot.trn_boot import _ntff_profile_via_ctypes

        hook = _ntff_profile_via_ctypes("/opt/axon/libaxon_pjrt.so")
    except Exception:
        hook = None
    mod = types.ModuleType("antenv.axon_hooks")
    mod._hook = hook
    mod.get_axon_ntff_profile_hook = lambda: mod._hook
    mod.set_axon_ntff_profile_hook = lambda h: setattr(mod, "_hook", h)
    sys.modules["antenv.axon_hooks"] = mod


_ensure_ntff_hook()

_NC = None


def _get_nc():
    global _NC
    if _NC is None:
        _NC = build_nc()
        # bacc passes legalize multi-sync-wait instructions for TRN2 codegen
        _NC.compile()
    return _NC


def make_in_maps(x, mu, logvar):
    x = np.ascontiguousarray(np.asarray(x, dtype=np.float32))
    mu = np.asarray(mu, dtype=np.float32)
    lv = np.asarray(logvar, dtype=np.float32)
    in_maps = []
    for c in range(N_CORES):
        r0 = c * ROWS
        mu_t = np.concatenate(
            [mu[r0 + b * HW:r0 + (b + 1) * HW].T for b in range(NB)], axis=0
        )
        lv_t = np.concatenate(
            [lv[r0 + b * HW:r0 + (b + 1) * HW].T for b in range(NB)], axis=0
        )
        x_t = x[c * NB:(c + 1) * NB].reshape(128, COLS)
        m = {"muf": np.ascontiguousarray(mu_t).astype(np.float16)}
        for h, s in enumerate(SL):
            m[f"lv{h}"] = np.ascontiguousarray(lv_t[:, s])
            m[f"x{h}"] = np.ascontiguousarray(x_t[:, s])
        in_maps.append(m)
    return in_maps


def combine(results) -> np.ndarray:
    tot = np.zeros((128, ACC_COLS), dtype=np.float64)
    for r in results:
        tot += np.asarray(r["accs"], dtype=np.float64)
    sinvv = tot[:, 0:2].sum(axis=1).reshape(NB, D).sum(axis=0)
    smuinvv = tot[:, 2:4].sum(axis=1).reshape(NB, D).sum(axis=0)
    sx2 = tot[:, 4:6].sum(axis=1).reshape(NB, D).sum(axis=0)
    sx = tot[:, 6:8].sum(axis=1).reshape(NB, D).sum(axis=0)
    A = tot[:, 8:10].sum()
    Bs = tot[:, 10:12].sum()
    loss = (-0.5 / N * (A - 2.0 * Bs)
            + 0.5 / N**2 * (sinvv @ sx2 - 2.0 * smuinvv @ sx))
    return np.array(loss, dtype=np.float32)


def kernel(x, mu, logvar, **_kwargs):
    nc = _get_nc()
    in_maps = make_in_maps(x, mu, logvar)
    res = run_bass_kernel_spmd(nc, in_maps, list(range(N_CORES)))
    return combine(res.results)


# revision 13
# speedup vs baseline: 1.1060x; 1.1060x over previous
"""CLUB loss kernel for Trainium2 (8 NeuronCores, SPMD row-sharded).

Math: the reference returns mean_i(pos_i - neg_i), a scalar.  Expanding
both terms, the C = sum mu^2*invv piece cancels exactly between pos and
neg, so the kernel only needs six fused reduction passes:

  loss = -0.5/N * (A - 2B) + 0.5/N^2 * (S_invv . S_x2 - 2 S_muinvv . S_x)
      A        = sum_{i,d} x^2 * invv        (scalar)
      B        = sum_{i,d} x * mu * invv     (scalar)
      S_invv   = sum_i invv[i,:]             (d-vector)
      S_muinvv = sum_i mu[i,:]*invv[i,:]     (d-vector)
      S_x      = sum_j x[j,:]                (d-vector)
      S_x2     = sum_j x[j,:]^2              (d-vector)

Each core handles 2048 rows (2 batches of x + matching mu/logvar rows)
and emits f32 partial sums; the host combines them in float64.

Layout: d-major (128, 1024): partition q = (sub-slab b, dim d), free
axis = row index.  Every reduction is a free-axis row-sum riding the
accum_out port of a fused elementwise op.

Schedule: asymmetric column halves a=[0:640], b=[640:1024].  mu ships
as ONE full-width fp16 tensor (halves the bytes at 2KB DMA lines;
quantization costs ~3e-3 end-to-end, gate is 2e-2).  Queues:
  sync:   lv_a, x_a          scalar: lv_b, mu, x_b      (640KB each)
ACT runs exp (small half first) + squares + Sx_a copy; DVE runs the
muinvv/B/A chain + Sx_b.  DVE junk outputs share one tile, WAW-pinning
their order against the scheduler's optimistic DMA model.
"""

import sys

sys.path.insert(0, "/opt/trn_rl_repo")

import numpy as np
from contextlib import ExitStack

import concourse.bass as bass
import concourse.bacc as bacc
import concourse.tile as tile
from concourse import mybir
from concourse.bass_utils import run_bass_kernel_spmd

F32 = mybir.dt.float32
F16 = mybir.dt.float16
BF16 = mybir.dt.bfloat16
N_CORES = 8
B, D, H, W = 16, 64, 32, 32
HW = H * W                # 1024
N = B * HW                # 16384
NB = B // N_CORES         # 2 sub-slabs (batches) per core
ROWS = NB * HW            # 2048 rows per core
COLS = HW                 # free size of the (128, 1024) layout

SPLIT = 512
SL = [slice(0, SPLIT), slice(SPLIT, COLS)]

# accumulator columns: [Sinvv a,b | Smuinvv a,b | Sx2 a,b | Sx a,b |
#                       A a,b | B a,b]
ACC_COLS = 12


def _col(acc, base, i):
    return acc[:, base + i:base + i + 1]


def build_nc() -> bass.Bass:
    nc = bacc.Bacc()
    lvt = [nc.dram_tensor(f"lv{h}", [128, s.stop - s.start], F32,
                          kind="ExternalInput") for h, s in enumerate(SL)]
    mut = nc.dram_tensor("muf", [128, COLS], F16, kind="ExternalInput")
    xt = [nc.dram_tensor(f"x{h}", [128, s.stop - s.start], F32,
                         kind="ExternalInput") for h, s in enumerate(SL)]
    accs = nc.dram_tensor("accs", [128, ACC_COLS], F32, kind="ExternalOutput")

    M = mybir.AluOpType.mult
    ADD = mybir.AluOpType.add
    BYP = mybir.AluOpType.bypass
    EXP = mybir.ActivationFunctionType.Exp
    SQ = mybir.ActivationFunctionType.Square
    CP = mybir.ActivationFunctionType.Copy

    with ExitStack() as ctx:
        tc = ctx.enter_context(tile.TileContext(nc))
        big = ctx.enter_context(tc.tile_pool(name="big", bufs=1))
        accp = ctx.enter_context(tc.tile_pool(name="accp", bufs=1))

        xb = big.tile([128, COLS], F32)
        mu = big.tile([128, COLS], F16)
        lv = big.tile([128, COLS], F32)
        invv = big.tile([128, COLS], F32)
        muinvv = big.tile([128, COLS], F32)
        x2 = big.tile([128, COLS], F32)
        jd = big.tile([128, SPLIT], BF16)   # shared junk out: WAW-chains DVE order
        ja = big.tile([128, SPLIT], BF16)   # ACT junk out for the Sx_a copy
        acc = accp.tile([128, ACC_COLS], F32)

        # sync: lv_a, mu(full, fp16), x_b — scalar: lv_b, x_a.
        # All lines are 2KB (512 f32 cols or 1024 fp16 cols): pow-2 lines
        # matter — a 640/384 split (2560B/1536B lines) ran ~2.5us slower.
        nc.sync.dma_start(out=lv[:, SL[0]], in_=lvt[0][:, :])
        nc.scalar.dma_start(out=lv[:, SL[1]], in_=lvt[1][:, :])
        nc.sync.dma_start(out=mu[:, :], in_=mut[:, :])
        nc.scalar.dma_start(out=xb[:, SL[0]], in_=xt[0][:, :])
        nc.sync.dma_start(out=xb[:, SL[1]], in_=xt[1][:, :])

        # ACT: exps, squares in x-arrival order, then the Sx_a copy.
        for h in (0, 1):
            s = SL[h]
            nc.scalar.activation(
                out=invv[:, s], in_=lv[:, s], func=EXP, bias=0.0, scale=-1.0,
                accum_out=_col(acc, 0, h),
            )
        for h in (0, 1):
            s = SL[h]
            nc.scalar.activation(
                out=x2[:, s], in_=xb[:, s], func=SQ, bias=0.0, scale=1.0,
                accum_out=_col(acc, 4, h),
            )
        nc.scalar.activation(
            out=ja[:], in_=xb[:, SL[0]], func=CP, bias=0.0, scale=1.0,
            accum_out=_col(acc, 6, 0),
        )

        # DVE: muinvv_a, B_a, muinvv_b, A_a, B_b, A_b, Sx_b.  The junk-out
        # ops share `jd` (WAW chain) and muinvv_b reads jd[:,0:1] as a
        # bypassed scalar (RAW on B_a) — both pin the order against the
        # scheduler's optimistic DMA model.
        nc.vector.scalar_tensor_tensor(
            out=muinvv[:, SL[0]], in0=mu[:, SL[0]], scalar=1.0,
            in1=invv[:, SL[0]], op0=M, op1=M, accum_out=_col(acc, 2, 0),
        )
        nc.vector.scalar_tensor_tensor(
            out=jd[:], in0=xb[:, SL[0]], scalar=1.0, in1=muinvv[:, SL[0]],
            op0=M, op1=M, accum_out=_col(acc, 10, 0),  # B_a
        )
        nc.vector.scalar_tensor_tensor(
            out=muinvv[:, SL[1]], in0=mu[:, SL[1]], scalar=jd[:, 0:1],
            in1=invv[:, SL[1]], op0=BYP, op1=M, accum_out=_col(acc, 2, 1),
        )
        nc.vector.scalar_tensor_tensor(
            out=jd[:], in0=x2[:, SL[0]], scalar=1.0, in1=invv[:, SL[0]],
            op0=M, op1=M, accum_out=_col(acc, 8, 0),   # A_a
        )
        nc.vector.scalar_tensor_tensor(
            out=jd[:], in0=xb[:, SL[1]], scalar=1.0, in1=muinvv[:, SL[1]],
            op0=M, op1=M, accum_out=_col(acc, 10, 1),  # B_b
        )
        nc.vector.scalar_tensor_tensor(
            out=jd[:], in0=x2[:, SL[1]], scalar=1.0, in1=invv[:, SL[1]],
            op0=M, op1=M, accum_out=_col(acc, 8, 1),   # A_b
        )
        nc.vector.tensor_scalar(
            out=jd[:], in0=xb[:, SL[1]], scalar1=1.0,
            scalar2=0.0, op0=M, op1=ADD, accum_out=_col(acc, 6, 1),  # Sx_b
        )

        nc.sync.dma_start(out=accs[:, :], in_=acc[:])
    return nc


def _ensure_ntff_hook():
    """This image's antenv lacks axon_hooks; if tracing is requested
    (e.g. BASS_TRACE=1), run_bass_kernel_spmd would die on the import.
    Register the ctypes-based hook if available, else a None hook so
    tracing degrades gracefully."""
    import types

    if "antenv.axon_hooks" in sys.modules:
        return
    try:
        import antenv.axon_hooks  # noqa: F401
        return
    except ImportError:
        pass
    hook = None
    try:
        sys.path.insert(0, "/root/.axon_site")
        from trn_agent_boot.trn_boot import _ntff_profile_via_ctypes

        hook = _ntff_profile_via_ctypes("/opt/axon/libaxon_pjrt.so")
    except Exception:
        hook = None
    mod = types.ModuleType("antenv.axon_hooks")
    mod._hook = hook
    mod.get_axon_ntff_profile_hook = lambda: mod._hook
    mod.set_axon_ntff_profile_hook = lambda h: setattr(mod, "_hook", h)
    sys.modules["antenv.axon_hooks"] = mod


_ensure_ntff_hook()

_NC = None


def _get_nc():
    global _NC
    if _NC is None:
        _NC = build_nc()
        # bacc passes legalize multi-sync-wait instructions for TRN2 codegen
        _NC.compile()
    return _NC


def make_in_maps(x, mu, logvar):
    x = np.ascontiguousarray(np.asarray(x, dtype=np.float32))
    mu = np.asarray(mu, dtype=np.float32)
    lv = np.asarray(logvar, dtype=np.float32)
    in_maps = []
    for c in range(N_CORES):
        r0 = c * ROWS
        mu_t = np.concatenate(
            [mu[r0 + b * HW:r0 + (b + 1) * HW].T for b in range(NB)], axis=0
        )
        lv_t = np.concatenate(
            [lv[r0 + b * HW:r0 + (b + 1) * HW].T for b in range(NB)], axis=0
        )
        x_t = x[c * NB:(c + 1) * NB].reshape(128, COLS)
        m = {"muf": np.ascontiguousarray(mu_t).astype(np.float16)}
        for h, s in enumerate(SL):
            m[f"lv{h}"] = np.ascontiguousarray(lv_t[:, s])
            m[f"x{h}"] = np.ascontiguousarray(x_t[:, s])
        in_maps.append(m)
    return in_maps


def combine(results) -> np.ndarray:
    tot = np.zeros((128, ACC_COLS), dtype=np.float64)
    for r in results:
        tot += np.asarray(r["accs"], dtype=np.float64)
    sinvv = tot[:, 0:2].sum(axis=1).reshape(NB, D).sum(axis=0)
    smuinvv = tot[:, 2:4].sum(axis=1).reshape(NB, D).sum(axis=0)
    sx2 = tot[:, 4:6].sum(axis=1).reshape(NB, D).sum(axis=0)
    sx = tot[:, 6:8].sum(axis=1).reshape(NB, D).sum(axis=0)
    A = tot[:, 8:10].sum()
    Bs = tot[:, 10:12].sum()
    loss = (-0.5 / N * (A - 2.0 * Bs)
            + 0.5 / N**2 * (sinvv @ sx2 - 2.0 * smuinvv @ sx))
    return np.array(loss, dtype=np.float32)


def kernel(x, mu, logvar, **_kwargs):
    nc = _get_nc()
    in_maps = make_in_maps(x, mu, logvar)
    res = run_bass_kernel_spmd(nc, in_maps, list(range(N_CORES)))
    return combine(res.results)


# revision 14
# speedup vs baseline: 1.1519x; 1.0415x over previous
"""CLUB loss kernel for Trainium2 (8 NeuronCores, SPMD row-sharded).

Math: the reference returns mean_i(pos_i - neg_i), a scalar.  Expanding
both terms, the C = sum mu^2*invv piece cancels exactly between pos and
neg, so the kernel only needs six fused reduction passes:

  loss = -0.5/N * (A - 2B) + 0.5/N^2 * (S_invv . S_x2 - 2 S_muinvv . S_x)
      A        = sum_{i,d} x^2 * invv        (scalar)
      B        = sum_{i,d} x * mu * invv     (scalar)
      S_invv   = sum_i invv[i,:]             (d-vector)
      S_muinvv = sum_i mu[i,:]*invv[i,:]     (d-vector)
      S_x      = sum_j x[j,:]                (d-vector)
      S_x2     = sum_j x[j,:]^2              (d-vector)

Each core handles 2048 rows (2 batches of x + matching mu/logvar rows)
and emits f32 partial sums; the host combines them in float64.

Layout: d-major (128, 1024): partition q = (sub-slab b, dim d), free
axis = row index.  Every reduction is a free-axis row-sum riding the
accum_out port of a fused elementwise op.

Schedule (sem-arrival times in comments are measured, not aspirational):
  sync:   lv[0:512] f32 | mu[0:64,:] fp16 | x[0:512] f32      (640KB)
  scalar: lv[512:]  f32 | mu[64:,:]  fp16 | x[512:]  f32      (640KB)
All DMA lines are 2KB (pow-2 lines matter: 2.5KB lines ran ~2.5us
slower; 1KB-line fp16 halves also regressed).  lv halves land ~10.8,
mu ~11.6, x ~12.9.  ACT: exp halves (chunked so muinvv can start
early), then full-width square and Sx copy.  DVE: muinvv halves, then
full-width B and A.  Full-width x-passes halve the per-op fixed costs
and the 279ns ACT accumulator reads.  mu ships fp16 (error ~3e-3
end-to-end vs the 2e-2 gate).
"""

import sys

sys.path.insert(0, "/opt/trn_rl_repo")

import numpy as np
from contextlib import ExitStack

import concourse.bass as bass
import concourse.bacc as bacc
import concourse.tile as tile
from concourse import mybir
from concourse.bass_utils import run_bass_kernel_spmd

F32 = mybir.dt.float32
F16 = mybir.dt.float16
BF16 = mybir.dt.bfloat16
N_CORES = 8
B, D, H, W = 16, 64, 32, 32
HW = H * W                # 1024
N = B * HW                # 16384
NB = B // N_CORES         # 2 sub-slabs (batches) per core
ROWS = NB * HW            # 2048 rows per core
COLS = HW                 # free size of the (128, 1024) layout

SPLIT = 512
SL = [slice(0, SPLIT), slice(SPLIT, COLS)]

# accumulator columns: [Sinvv a,b | Smuinvv a,b | Sx2 | Sx | A | B]
ACC_COLS = 8


def build_nc() -> bass.Bass:
    nc = bacc.Bacc()
    lvt = [nc.dram_tensor(f"lv{h}", [128, s.stop - s.start], F32,
                          kind="ExternalInput") for h, s in enumerate(SL)]
    mut = [nc.dram_tensor(f"mup{p}", [64, COLS], F16, kind="ExternalInput")
           for p in range(2)]
    xt = [nc.dram_tensor(f"x{h}", [128, s.stop - s.start], F32,
                         kind="ExternalInput") for h, s in enumerate(SL)]
    accs = nc.dram_tensor("accs", [128, ACC_COLS], F32, kind="ExternalOutput")

    M = mybir.AluOpType.mult
    EXP = mybir.ActivationFunctionType.Exp
    SQ = mybir.ActivationFunctionType.Square
    CP = mybir.ActivationFunctionType.Copy

    with ExitStack() as ctx:
        tc = ctx.enter_context(tile.TileContext(nc))
        big = ctx.enter_context(tc.tile_pool(name="big", bufs=1))
        accp = ctx.enter_context(tc.tile_pool(name="accp", bufs=1))

        xb = big.tile([128, COLS], F32)
        mu = big.tile([128, COLS], F16)
        lv = big.tile([128, COLS], F32)
        invv = big.tile([128, COLS], F32)
        muinvv = big.tile([128, COLS], F32)
        x2 = big.tile([128, COLS], F32)
        jd = big.tile([128, COLS], BF16)   # shared junk out pins DVE order
        ja = big.tile([128, COLS], BF16)   # ACT junk out for the Sx copy
        acc = accp.tile([128, ACC_COLS], F32)

        nc.sync.dma_start(out=lv[:, SL[0]], in_=lvt[0][:, :])
        nc.scalar.dma_start(out=lv[:, SL[1]], in_=lvt[1][:, :])
        nc.sync.dma_start(out=mu[0:64, :], in_=mut[0][:, :])
        nc.scalar.dma_start(out=mu[64:128, :], in_=mut[1][:, :])
        nc.sync.dma_start(out=xb[:, SL[0]], in_=xt[0][:, :])
        nc.scalar.dma_start(out=xb[:, SL[1]], in_=xt[1][:, :])

        # ACT: exp halves, then full-width square + Sx copy.
        for h in (0, 1):
            s = SL[h]
            nc.scalar.activation(
                out=invv[:, s], in_=lv[:, s], func=EXP, bias=0.0, scale=-1.0,
                accum_out=acc[:, h:h + 1],
            )
        nc.scalar.activation(
            out=x2[:], in_=xb[:], func=SQ, bias=0.0, scale=1.0,
            accum_out=acc[:, 4:5],
        )
        nc.scalar.activation(
            out=ja[:], in_=xb[:], func=CP, bias=0.0, scale=1.0,
            accum_out=acc[:, 5:6],
        )

        # DVE: muinvv halves (start as soon as invv_a + mu land), then
        # full-width B and A (jd WAW keeps B before A).
        for h in (0, 1):
            s = SL[h]
            nc.vector.scalar_tensor_tensor(
                out=muinvv[:, s], in0=mu[:, s], scalar=1.0, in1=invv[:, s],
                op0=M, op1=M, accum_out=acc[:, 2 + h:3 + h],
            )
        nc.vector.scalar_tensor_tensor(
            out=jd[:], in0=xb[:], scalar=1.0, in1=muinvv[:],
            op0=M, op1=M, accum_out=acc[:, 7:8],   # B
        )
        nc.vector.scalar_tensor_tensor(
            out=jd[:], in0=x2[:], scalar=1.0, in1=invv[:],
            op0=M, op1=M, accum_out=acc[:, 6:7],   # A
        )

        nc.sync.dma_start(out=accs[:, :], in_=acc[:])
    return nc


def _ensure_ntff_hook():
    """This image's antenv lacks axon_hooks; if tracing is requested
    (e.g. BASS_TRACE=1), run_bass_kernel_spmd would die on the import.
    Register the ctypes-based hook if available, else a None hook so
    tracing degrades gracefully."""
    import types

    if "antenv.axon_hooks" in sys.modules:
        return
    try:
        import antenv.axon_hooks  # noqa: F401
        return
    except ImportError:
        pass
    hook = None
    try:
        sys.path.insert(0, "/root/.axon_site")
        from trn_agent_boot.trn_boot import _ntff_profile_via_ctypes

        hook = _ntff_profile_via_ctypes("/opt/axon/libaxon_pjrt.so")
    except Exception:
        hook = None
    mod = types.ModuleType("antenv.axon_hooks")
    mod._hook = hook
    mod.get_axon_ntff_profile_hook = lambda: mod._hook
    mod.set_axon_ntff_profile_hook = lambda h: setattr(mod, "_hook", h)
    sys.modules["antenv.axon_hooks"] = mod


_ensure_ntff_hook()

_NC = None


def _get_nc():
    global _NC
    if _NC is None:
        _NC = build_nc()
        # bacc passes legalize multi-sync-wait instructions for TRN2 codegen
        _NC.compile()
    return _NC


def make_in_maps(x, mu, logvar):
    x = np.ascontiguousarray(np.asarray(x, dtype=np.float32))
    mu = np.asarray(mu, dtype=np.float32)
    lv = np.asarray(logvar, dtype=np.float32)
    in_maps = []
    for c in range(N_CORES):
        r0 = c * ROWS
        mu_t = np.concatenate(
            [mu[r0 + b * HW:r0 + (b + 1) * HW].T for b in range(NB)], axis=0
        ).astype(np.float16)
        lv_t = np.concatenate(
            [lv[r0 + b * HW:r0 + (b + 1) * HW].T for b in range(NB)], axis=0
        )
        x_t = x[c * NB:(c + 1) * NB].reshape(128, COLS)
        m = {"mup0": np.ascontiguousarray(mu_t[0:64]),
             "mup1": np.ascontiguousarray(mu_t[64:128])}
        for h, s in enumerate(SL):
            m[f"lv{h}"] = np.ascontiguousarray(lv_t[:, s])
            m[f"x{h}"] = np.ascontiguousarray(x_t[:, s])
        in_maps.append(m)
    return in_maps


def combine(results) -> np.ndarray:
    tot = np.zeros((128, ACC_COLS), dtype=np.float64)
    for r in results:
        tot += np.asarray(r["accs"], dtype=np.float64)
    sinvv = tot[:, 0:2].sum(axis=1).reshape(NB, D).sum(axis=0)
    smuinvv = tot[:, 2:4].sum(axis=1).reshape(NB, D).sum(axis=0)
    sx2 = tot[:, 4].reshape(NB, D).sum(axis=0)
    sx = tot[:, 5].reshape(NB, D).sum(axis=0)
    A = tot[:, 6].sum()
    Bs = tot[:, 7].sum()
    loss = (-0.5 / N * (A - 2.0 * Bs)
            + 0.5 / N**2 * (sinvv @ sx2 - 2.0 * smuinvv @ sx))
    return np.array(loss, dtype=np.float32)


def kernel(x, mu, logvar, **_kwargs):
    nc = _get_nc()
    in_maps = make_in_maps(x, mu, logvar)
    res = run_bass_kernel_spmd(nc, in_maps, list(range(N_CORES)))
    return combine(res.results)
